# revision 21
# baseline (speedup 1.0000x reference)
"""Sparse (causal + kv-padding) attention on 8 TRN2 NeuronCores via Bass/Tile.

Shapes (hardcoded per spec): B=2, H=16, S=2048, D=64, fp32.
Sharding: batch*head (32 pairs) split 4-per-core across 8 cores; no collectives.

Per-core algorithm (per head):
  S^T[kv, q] = K @ Q^T           (TensorE, contraction d=64, kv-tiles row-packed 2x)
  P^T = exp(S^T * scale)         (ScalarE, PSUM->SBUF, scale folded into activation)
  causal diag tiles: P^T *= upper-tri 0/1 mask (VectorE)
  kv padding: folded into V_aug = [V*kvmask | kvmask] host-side, so masked kv
              contribute 0 to both O_unnorm and the softmax denominator s.
  O^T_aug[65, q] = V_aug^T @ P^T (TensorE, accumulated over kv tiles in PSUM;
                                  row 64 = s = sum_kv P^T)
  O = (O^T[0:64] / s).T          (reciprocal on VectorE, transpose via TensorE,
                                  per-partition tensor_scalar multiply)
No softmax max-subtraction: logits are ~N(0,1) after scaling, exp is fp32-safe.
"""

import math
import os
from contextlib import ExitStack

import numpy as np

import concourse.bass as bass
import concourse.mybir as mybir
import concourse.tile as tile
from concourse import bacc
from concourse.bass_utils import run_bass_kernel_spmd
from concourse.masks import make_identity

B, H, S, D = 2, 16, 2048, 64
N_CORES = 8
HPC = (B * H) // N_CORES  # heads per core = 4
NKV = S // 128            # 16 kv tiles per head
QB = 512                  # q block width (PSUM bank)
NQB = S // QB             # 4 q blocks
KVPB = QB // 128          # kv tiles per q block = 4
SCALE = 1.0 / math.sqrt(D)
F32 = mybir.dt.float32
FULL_GROUP = 3            # kv tiles per S^T psum group (3 banks)
# matmul input dtype: "fp32" (2-pass LOW_HIGH, slow), "fp32r" (1-pass TF32-ish),
# "bf16" (1-pass, 2x col rate, ~2e-3 rel err)
MM_DT = os.environ.get("ATTN_MM_DT", "fp32r")
DT_IN = {"fp32": F32, "fp32r": mybir.dt.float32r,
         "bf16": mybir.dt.bfloat16}[MM_DT]


def np_in_dtype():
    return mybir.dt.np(DT_IN)


def _mm(ap):
    return ap

# stash for test harness introspection (exec_time_ns etc.)
last_results = None


def _build_program():
    nc = bacc.Bacc("TRN2", target_bir_lowering=False, debug=False,
                   num_devices=N_CORES)
    qt_d = nc.dram_tensor("qt", [HPC, 128, S], DT_IN, kind="ExternalInput")
    kt_d = nc.dram_tensor("kt", [HPC, 128, NKV // 2, 128], DT_IN,
                          kind="ExternalInput")
    va_d = nc.dram_tensor("va", [HPC, 128, NKV, D + 1], DT_IN,
                          kind="ExternalInput")
    # additive causal mask: 0 where kv<=q else -1e30 (applied pre-exp on PSUM)
    utm_d = nc.dram_tensor("utm", [128, 128], F32, kind="ExternalInput")
    out_d = nc.dram_tensor("out", [HPC, S, D], F32, kind="ExternalOutput")

    with ExitStack() as ctx:
        tc = ctx.enter_context(tile.TileContext(nc))
        const_pool = ctx.enter_context(tc.tile_pool(name="const", bufs=1))
        qt_pool = ctx.enter_context(tc.tile_pool(name="qtp", bufs=2))
        kt_pool = ctx.enter_context(tc.tile_pool(name="ktp", bufs=2))
        va_pool = ctx.enter_context(tc.tile_pool(name="vap", bufs=2))
        pt_pool = ctx.enter_context(tc.tile_pool(name="ptp", bufs=4))
        ot_pool = ctx.enter_context(tc.tile_pool(name="otp", bufs=2))
        sresh_pool = ctx.enter_context(tc.tile_pool(name="srp", bufs=2))
        rt_pool = ctx.enter_context(tc.tile_pool(name="rtp", bufs=2))
        outsb_pool = ctx.enter_context(tc.tile_pool(name="osp", bufs=2))
        sps_pool = ctx.enter_context(tc.tile_pool(name="sps", bufs=2,
                                                  space="PSUM"))
        oacc_pool = ctx.enter_context(tc.tile_pool(name="oac", bufs=1,
                                                   space="PSUM"))
        tp_pool = ctx.enter_context(tc.tile_pool(name="tpp", bufs=1,
                                                 space="PSUM"))

        utm = const_pool.tile([128, 128], F32)
        nc.sync.dma_start(utm[:, :], utm_d[:, :])
        ident = const_pool.tile([64, 64], F32)
        make_identity(nc, ident[:, :])

        for hl in range(HPC):
            qt = qt_pool.tile([128, S], DT_IN, tag="qt")
            nc.sync.dma_start(qt[:, :], qt_d[hl])
            kt = kt_pool.tile([128, NKV // 2, 128], DT_IN, tag="kt")
            nc.sync.dma_start(kt[:, :, :], kt_d[hl])
            va = va_pool.tile([128, NKV, D + 1], DT_IN, tag="va")
            nc.sync.dma_start(va[:, :, :], va_d[hl])

            ot = ot_pool.tile([65, S], F32, tag="ot")  # O^T_aug for the head
            outsb = outsb_pool.tile([128, S // 128, D], F32, tag="outsb")

            for qb in range(NQB):
                oacc = oacc_pool.tile([65, QB], F32, tag="oacc")
                q0 = qb * QB
                diag0 = KVPB * qb  # first diagonal kv tile

                # Build groups: full kv tiles [0, diag0) in chunks of 3,
                # then the 4 diagonal tiles packed into one group.
                groups = []  # (kind, [(j, psum_col, width, qoff), ...])
                full = list(range(diag0))
                for g0 in range(0, len(full), FULL_GROUP):
                    chunk = full[g0:g0 + FULL_GROUP]
                    groups.append(("full", [(j, 512 * k, 512, 0)
                                            for k, j in enumerate(chunk)]))
                # diag tiles t=0..3: widths 512,384,256,128, q offsets 128*t
                # packed at psum cols: t0 [0:512], t1 [512:896],
                # t2 [1024:1280], t3 [896:1024]  (each within one bank)
                groups.append(("diag", [
                    (diag0 + 0, 0, 512, 0),
                    (diag0 + 1, 512, 384, 128),
                    (diag0 + 2, 1024, 256, 256),
                    (diag0 + 3, 896, 128, 384),
                ]))

                for kind, items in groups:
                    s_ps = sps_pool.tile([128, 3 * 512], F32, tag="sps")
                    width = max(c + w for _, c, w, _ in items)
                    # QK^T matmuls; even kv tiles use array rows 0-63,
                    # odd tiles rows 64-127 (concurrent row-tiled pairs).
                    for j, pcol, w, qoff in items:
                        lo, hi = (0, 64) if j % 2 == 0 else (64, 128)
                        nc.tensor.matmul(
                            s_ps[:, pcol:pcol + w],
                            _mm(kt[lo:hi, j // 2, :]),
                            _mm(qt[lo:hi, q0 + qoff:q0 + QB]),
                            start=True, stop=True,
                        )
                    if kind == "diag":
                        # additive causal mask on the 4 diagonal blocks,
                        # applied on PSUM before exp (keeps exp the sole
                        # writer of pt -> fewer cross-engine waits on PE)
                        for _, pcol, _, _ in items:
                            nc.vector.tensor_add(s_ps[:, pcol:pcol + 128],
                                                 s_ps[:, pcol:pcol + 128],
                                                 utm[:, :])
                    pt = pt_pool.tile([128, 3 * 512], DT_IN, tag="pt")
                    nc.scalar.activation(pt[:, :width], s_ps[:, :width],
                                         mybir.ActivationFunctionType.Exp,
                                         scale=SCALE)
                    # PV: O^T_aug[65, q] += V_aug_j^T @ P^T_j
                    last_j = diag0 + KVPB - 1
                    for j, pcol, w, qoff in items:
                        nc.tensor.matmul(
                            oacc[:, qoff:QB],
                            _mm(va[:, j, :]),
                            _mm(pt[:, pcol:pcol + w]),
                            start=(j == 0), stop=(j == last_j),
                        )

                # evacuate O^T_aug block to SBUF
                nc.vector.tensor_copy(ot[:, q0:q0 + QB], oacc[:, :])

                # s chunk [1, 512] -> [4, 128] (SBUF->SBUF dma reshape)
                sresh = sresh_pool.tile([4, 128], F32, tag="sresh")
                nc.sync.dma_start(sresh[:, :], ot[64:65, q0:q0 + QB])
                # transpose to [128, 4] and reciprocal
                st_ps = tp_pool.tile([128, 64], F32, tag="tp")
                nc.tensor.transpose(st_ps[:, 0:4], sresh[:, :],
                                    ident[0:4, 0:4])
                rt = rt_pool.tile([128, 4], F32, tag="rt")
                nc.vector.reciprocal(rt[:, :], st_ps[:, 0:4])

                # final: transpose O^T 128-col chunks back to [q, d] and scale
                for c in range(4):
                    ot_ps = tp_pool.tile([128, 64], F32, tag="tp")
                    nc.tensor.transpose(
                        ot_ps[:, :],
                        ot[0:64, q0 + 128 * c:q0 + 128 * (c + 1)],
                        ident[:, :])
                    nc.vector.tensor_scalar_mul(outsb[:, 4 * qb + c, :],
                                                ot_ps[:, :], rt[:, c:c + 1])

            # one output DMA per head: outsb[p, m, d] -> out[128m+p, d]
            nc.sync.dma_start(
                out_d[hl].rearrange("(m p) d -> p m d", p=128),
                outsb[:, :, :])
    nc.compile()
    return nc


_program_cache = None


def _get_program():
    global _program_cache
    if _program_cache is None:
        _program_cache = _build_program()
    return _program_cache


def kernel(**inputs):
    q = np.asarray(inputs["query_states"], dtype=np.float32)
    k = np.asarray(inputs["key_states"], dtype=np.float32)
    v = np.asarray(inputs["value_states"], dtype=np.float32)
    kvm = np.asarray(inputs["kv_sequence_mask"])

    qf = q.reshape(B * H, S, D)
    kf = k.reshape(B * H, S, D)
    vf = v.reshape(B * H, S, D)
    # additive causal mask: 0 where kv<=q (keep), -1e30 where kv>q (mask)
    utm = np.where(np.triu(np.ones((128, 128), dtype=bool)), 0.0,
                   -1e30).astype(np.float32)

    in_maps = []
    for c in range(N_CORES):
        hs = slice(c * HPC, (c + 1) * HPC)
        b = (c * HPC) // H  # all heads of a core belong to one batch elem

        qt_c = qf[hs].transpose(0, 2, 1)                   # [4, 64, 2048]
        qt_c = np.concatenate([qt_c, qt_c], axis=1)        # [4, 128, 2048]

        kt_t = kf[hs].transpose(0, 2, 1).reshape(HPC, 64, NKV, 128)
        kt_c = np.concatenate([kt_t[:, :, 0::2, :],
                               kt_t[:, :, 1::2, :]], axis=1)  # [4,128,8,128]

        bmask = kvm[b].astype(np.float32)                  # [S]
        va_c = np.empty((HPC, S, D + 1), dtype=np.float32)
        va_c[:, :, :D] = vf[hs] * bmask[None, :, None]
        va_c[:, :, D] = bmask[None, :]
        va_c = va_c.reshape(HPC, NKV, 128, D + 1).transpose(0, 2, 1, 3)

        npdt = np_in_dtype()
        in_maps.append({
            "qt": np.ascontiguousarray(qt_c).astype(npdt),
            "kt": np.ascontiguousarray(kt_c).astype(npdt),
            "va": np.ascontiguousarray(va_c).astype(npdt),
            "utm": utm,
        })

    nc = _get_program()
    trace = bool(int(os.environ.get("ATTN_TRACE", "0")))
    res = run_bass_kernel_spmd(nc, in_maps, core_ids=list(range(N_CORES)),
                               trace=trace)
    global last_results
    last_results = res

    outs = np.stack([r["out"] for r in res.results])       # [8, 4, S, D]
    attn = outs.reshape(B, H, S, D)
    return (attn, np.asarray(inputs["key_states"]),
            np.asarray(inputs["value_states"]))


# revision 26
# speedup vs baseline: 1.1654x; 1.1654x over previous
"""Sparse (causal + kv-padding) attention on 8 TRN2 NeuronCores via Bass/Tile.

Shapes (hardcoded per spec): B=2, H=16, S=2048, D=64, fp32.
Sharding: batch*head (32 pairs) split 4-per-core across 8 cores; no collectives.

Per-core algorithm (per head):
  S^T[kv, q] = K @ Q^T           (TensorE, contraction d=64, kv-tiles row-packed 2x)
  P^T = exp(S^T * scale)         (ScalarE, PSUM->SBUF, scale folded into activation)
  causal diag tiles: P^T *= upper-tri 0/1 mask (VectorE)
  kv padding: folded into V_aug = [V*kvmask | kvmask] host-side, so masked kv
              contribute 0 to both O_unnorm and the softmax denominator s.
  O^T_aug[65, q] = V_aug^T @ P^T (TensorE, accumulated over kv tiles in PSUM;
                                  row 64 = s = sum_kv P^T)
  O = (O^T[0:64] / s).T          (reciprocal on VectorE, transpose via TensorE,
                                  per-partition tensor_scalar multiply)
No softmax max-subtraction: logits are ~N(0,1) after scaling, exp is fp32-safe.
"""

import math
import os
from contextlib import ExitStack

import numpy as np

import concourse.bass as bass
import concourse.mybir as mybir
import concourse.tile as tile
from concourse import bacc
from concourse.bass_utils import run_bass_kernel_spmd
from concourse.masks import make_identity

B, H, S, D = 2, 16, 2048, 64
N_CORES = 8
HPC = (B * H) // N_CORES  # heads per core = 4
NKV = S // 128            # 16 kv tiles per head
QB = 512                  # q block width (PSUM bank)
NQB = S // QB             # 4 q blocks
KVPB = QB // 128          # kv tiles per q block = 4
SCALE = 1.0 / math.sqrt(D)
F32 = mybir.dt.float32
FULL_GROUP = 3            # kv tiles per S^T psum group (3 banks)
# matmul input dtype: "fp32" (2-pass LOW_HIGH, slow), "fp32r" (1-pass TF32-ish),
# "bf16" (1-pass, 2x col rate, ~2e-3 rel err)
MM_DT = os.environ.get("ATTN_MM_DT", "fp32r")
DT_IN = {"fp32": F32, "fp32r": mybir.dt.float32r,
         "bf16": mybir.dt.bfloat16}[MM_DT]


def np_in_dtype():
    return mybir.dt.np(DT_IN)


def _mm(ap):
    return ap

# stash for test harness introspection (exec_time_ns etc.)
last_results = None


def _build_program():
    nc = bacc.Bacc("TRN2", target_bir_lowering=False, debug=False,
                   num_devices=N_CORES)
    qt_d = nc.dram_tensor("qt", [HPC, 128, S], DT_IN, kind="ExternalInput")
    kt_d = nc.dram_tensor("kt", [HPC, 128, NKV // 2, 128], DT_IN,
                          kind="ExternalInput")
    va_d = nc.dram_tensor("va", [HPC, 128, NKV, D + 1], DT_IN,
                          kind="ExternalInput")
    utm_d = nc.dram_tensor("utm", [128, 128], DT_IN, kind="ExternalInput")
    out_d = nc.dram_tensor("out", [HPC, S, D], F32, kind="ExternalOutput")

    with ExitStack() as ctx:
        tc = ctx.enter_context(tile.TileContext(nc))
        const_pool = ctx.enter_context(tc.tile_pool(name="const", bufs=1))
        qt_pool = ctx.enter_context(tc.tile_pool(name="qtp", bufs=2))
        kt_pool = ctx.enter_context(tc.tile_pool(name="ktp", bufs=2))
        va_pool = ctx.enter_context(tc.tile_pool(name="vap", bufs=2))
        pt_pool = ctx.enter_context(tc.tile_pool(name="ptp", bufs=4))
        ot_pool = ctx.enter_context(tc.tile_pool(name="otp", bufs=2))
        sresh_pool = ctx.enter_context(tc.tile_pool(name="srp", bufs=2))
        rt_pool = ctx.enter_context(tc.tile_pool(name="rtp", bufs=2))
        outsb_pool = ctx.enter_context(tc.tile_pool(name="osp", bufs=2))
        sps_pool = ctx.enter_context(tc.tile_pool(name="sps", bufs=2,
                                                  space="PSUM"))
        oacc_pool = ctx.enter_context(tc.tile_pool(name="oac", bufs=1,
                                                   space="PSUM"))
        tp_pool = ctx.enter_context(tc.tile_pool(name="tpp", bufs=1,
                                                 space="PSUM"))

        utm = const_pool.tile([128, 128], DT_IN)
        nc.sync.dma_start(utm[:, :], utm_d[:, :])
        ident = const_pool.tile([64, 64], F32)
        make_identity(nc, ident[:, :])

        for hl in range(HPC):
            qt = qt_pool.tile([128, S], DT_IN, tag="qt")
            nc.sync.dma_start(qt[:, :], qt_d[hl])
            kt = kt_pool.tile([128, NKV // 2, 128], DT_IN, tag="kt")
            nc.sync.dma_start(kt[:, :, :], kt_d[hl])
            va = va_pool.tile([128, NKV, D + 1], DT_IN, tag="va")
            nc.sync.dma_start(va[:, :, :], va_d[hl])

            ot = ot_pool.tile([65, S], F32, tag="ot")  # O^T_aug for the head
            outsb = outsb_pool.tile([128, S // 128, D], F32, tag="outsb")

            for qb in range(NQB):
                oacc = oacc_pool.tile([65, QB], F32, tag="oacc")
                q0 = qb * QB
                diag0 = KVPB * qb  # first diagonal kv tile

                # Build groups: full kv tiles [0, diag0) in chunks of 3,
                # then the 4 diagonal tiles packed into one group.
                groups = []  # (kind, [(j, psum_col, width, qoff), ...])
                full = list(range(diag0))
                for g0 in range(0, len(full), FULL_GROUP):
                    chunk = full[g0:g0 + FULL_GROUP]
                    groups.append(("full", [(j, 512 * k, 512, 0)
                                            for k, j in enumerate(chunk)]))
                # diag tiles t=0..3: widths 512,384,256,128, q offsets 128*t
                # packed at psum cols: t0 [0:512], t1 [512:896],
                # t2 [1024:1280], t3 [896:1024]  (each within one bank)
                groups.append(("diag", [
                    (diag0 + 0, 0, 512, 0),
                    (diag0 + 1, 512, 384, 128),
                    (diag0 + 2, 1024, 256, 256),
                    (diag0 + 3, 896, 128, 384),
                ]))

                for kind, items in groups:
                    s_ps = sps_pool.tile([128, 3 * 512], F32, tag="sps")
                    width = max(c + w for _, c, w, _ in items)
                    # QK^T matmuls; even kv tiles use array rows 0-63,
                    # odd tiles rows 64-127 (concurrent row-tiled pairs).
                    for j, pcol, w, qoff in items:
                        lo, hi = (0, 64) if j % 2 == 0 else (64, 128)
                        nc.tensor.matmul(
                            s_ps[:, pcol:pcol + w],
                            _mm(kt[lo:hi, j // 2, :]),
                            _mm(qt[lo:hi, q0 + qoff:q0 + QB]),
                            start=True, stop=True,
                        )
                    pt = pt_pool.tile([128, 3 * 512], DT_IN, tag="pt")
                    nc.scalar.activation(pt[:, :width], s_ps[:, :width],
                                         mybir.ActivationFunctionType.Exp,
                                         scale=SCALE)
                    if kind == "diag":
                        # triangular causal mask on the 4 diagonal blocks
                        for _, pcol, _, _ in items:
                            nc.vector.tensor_mul(pt[:, pcol:pcol + 128],
                                                 pt[:, pcol:pcol + 128],
                                                 utm[:, :])
                    # PV: O^T_aug[65, q] += V_aug_j^T @ P^T_j
                    last_j = diag0 + KVPB - 1
                    for j, pcol, w, qoff in items:
                        nc.tensor.matmul(
                            oacc[:, qoff:QB],
                            _mm(va[:, j, :]),
                            _mm(pt[:, pcol:pcol + w]),
                            start=(j == 0), stop=(j == last_j),
                        )

                # evacuate O^T_aug block to SBUF
                nc.vector.tensor_copy(ot[:, q0:q0 + QB], oacc[:, :])

                # s chunk [1, 512] -> [4, 128] (SBUF->SBUF dma reshape)
                sresh = sresh_pool.tile([4, 128], F32, tag="sresh")
                nc.sync.dma_start(sresh[:, :], ot[64:65, q0:q0 + QB])
                # transpose to [128, 4] and reciprocal
                st_ps = tp_pool.tile([128, 64], F32, tag="tp")
                nc.tensor.transpose(st_ps[:, 0:4], sresh[:, :],
                                    ident[0:4, 0:4])
                rt = rt_pool.tile([128, 4], F32, tag="rt")
                nc.vector.reciprocal(rt[:, :], st_ps[:, 0:4])

                # final: transpose O^T 128-col chunks back to [q, d] and scale
                for c in range(4):
                    ot_ps = tp_pool.tile([128, 64], F32, tag="tp")
                    nc.tensor.transpose(
                        ot_ps[:, :],
                        ot[0:64, q0 + 128 * c:q0 + 128 * (c + 1)],
                        ident[:, :])
                    nc.vector.tensor_scalar_mul(outsb[:, 4 * qb + c, :],
                                                ot_ps[:, :], rt[:, c:c + 1])

            # one output DMA per head: outsb[p, m, d] -> out[128m+p, d]
            nc.sync.dma_start(
                out_d[hl].rearrange("(m p) d -> p m d", p=128),
                outsb[:, :, :])
    nc.compile()
    return nc


_program_cache = None


def _get_program():
    global _program_cache
    if _program_cache is None:
        _program_cache = _build_program()
    return _program_cache


def kernel(**inputs):
    q = np.asarray(inputs["query_states"], dtype=np.float32)
    k = np.asarray(inputs["key_states"], dtype=np.float32)
    v = np.asarray(inputs["value_states"], dtype=np.float32)
    kvm = np.asarray(inputs["kv_sequence_mask"])

    qf = q.reshape(B * H, S, D)
    kf = k.reshape(B * H, S, D)
    vf = v.reshape(B * H, S, D)
    utm = np.triu(np.ones((128, 128), dtype=np.float32))  # keep kv<=q

    in_maps = []
    for c in range(N_CORES):
        hs = slice(c * HPC, (c + 1) * HPC)
        b = (c * HPC) // H  # all heads of a core belong to one batch elem

        qt_c = qf[hs].transpose(0, 2, 1)                   # [4, 64, 2048]
        qt_c = np.concatenate([qt_c, qt_c], axis=1)        # [4, 128, 2048]

        kt_t = kf[hs].transpose(0, 2, 1).reshape(HPC, 64, NKV, 128)
        kt_c = np.concatenate([kt_t[:, :, 0::2, :],
                               kt_t[:, :, 1::2, :]], axis=1)  # [4,128,8,128]

        bmask = kvm[b].astype(np.float32)                  # [S]
        va_c = np.empty((HPC, S, D + 1), dtype=np.float32)
        va_c[:, :, :D] = vf[hs] * bmask[None, :, None]
        va_c[:, :, D] = bmask[None, :]
        va_c = va_c.reshape(HPC, NKV, 128, D + 1).transpose(0, 2, 1, 3)

        npdt = np_in_dtype()
        in_maps.append({
            "qt": np.ascontiguousarray(qt_c).astype(npdt),
            "kt": np.ascontiguousarray(kt_c).astype(npdt),
            "va": np.ascontiguousarray(va_c).astype(npdt),
            "utm": utm.astype(npdt),
        })

    nc = _get_program()
    trace = bool(int(os.environ.get("ATTN_TRACE", "0")))
    res = run_bass_kernel_spmd(nc, in_maps, core_ids=list(range(N_CORES)),
                               trace=trace)
    global last_results
    last_results = res

    outs = np.stack([r["out"] for r in res.results])       # [8, 4, S, D]
    attn = outs.reshape(B, H, S, D)
    return (attn, np.asarray(inputs["key_states"]),
            np.asarray(inputs["value_states"]))


# revision 27
# speedup vs baseline: 1.2247x; 1.0509x over previous
"""Sparse (causal + kv-padding) attention on 8 TRN2 NeuronCores via Bass/Tile.

Shapes (hardcoded per spec): B=2, H=16, S=2048, D=64, fp32.
Sharding: batch*head (32 pairs) split 4-per-core across 8 cores; no collectives.

Per-core algorithm (per head):
  S^T[kv, q] = K @ Q^T           (TensorE, contraction d=64, kv-tiles row-packed 2x)
  P^T = exp(S^T * scale)         (ScalarE, PSUM->SBUF, scale folded into activation)
  causal diag tiles: P^T *= upper-tri 0/1 mask (VectorE)
  kv padding: folded into V_aug = [V*kvmask | kvmask] host-side, so masked kv
              contribute 0 to both O_unnorm and the softmax denominator s.
  O^T_aug[65, q] = V_aug^T @ P^T (TensorE, accumulated over kv tiles in PSUM;
                                  row 64 = s = sum_kv P^T)
  O = (O^T[0:64] / s).T          (reciprocal on VectorE, transpose via TensorE,
                                  per-partition tensor_scalar multiply)
No softmax max-subtraction: logits are ~N(0,1) after scaling, exp is fp32-safe.
"""

import math
import os
from contextlib import ExitStack

import numpy as np

import concourse.bass as bass
import concourse.mybir as mybir
import concourse.tile as tile
from concourse import bacc
from concourse.bass_utils import run_bass_kernel_spmd
from concourse.masks import make_identity

B, H, S, D = 2, 16, 2048, 64
N_CORES = 8
HPC = (B * H) // N_CORES  # heads per core = 4
NKV = S // 128            # 16 kv tiles per head
QB = 512                  # q block width (PSUM bank)
NQB = S // QB             # 4 q blocks
KVPB = QB // 128          # kv tiles per q block = 4
SCALE = 1.0 / math.sqrt(D)
F32 = mybir.dt.float32
FULL_GROUP = 3            # kv tiles per S^T psum group (3 banks)
# matmul input dtype: "fp32" (2-pass LOW_HIGH, slow), "fp32r" (1-pass TF32-ish),
# "bf16" (1-pass, 2x col rate, ~2e-3 rel err)
MM_DT = os.environ.get("ATTN_MM_DT", "fp32r")
DT_IN = {"fp32": F32, "fp32r": mybir.dt.float32r,
         "bf16": mybir.dt.bfloat16}[MM_DT]


def np_in_dtype():
    return mybir.dt.np(DT_IN)


def _mm(ap):
    return ap

# stash for test harness introspection (exec_time_ns etc.)
last_results = None


def _build_program():
    nc = bacc.Bacc("TRN2", target_bir_lowering=False, debug=False,
                   num_devices=N_CORES)
    qt_d = nc.dram_tensor("qt", [HPC, 128, S], DT_IN, kind="ExternalInput")
    kt_d = nc.dram_tensor("kt", [HPC, 128, NKV // 2, 128], DT_IN,
                          kind="ExternalInput")
    va_d = nc.dram_tensor("va", [HPC, 128, NKV, D + 1], DT_IN,
                          kind="ExternalInput")
    utm_d = nc.dram_tensor("utm", [128, 128], DT_IN, kind="ExternalInput")
    out_d = nc.dram_tensor("out", [HPC, S, D], F32, kind="ExternalOutput")

    with ExitStack() as ctx:
        tc = ctx.enter_context(tile.TileContext(nc))
        const_pool = ctx.enter_context(tc.tile_pool(name="const", bufs=1))
        qt_pool = ctx.enter_context(tc.tile_pool(name="qtp", bufs=2))
        kt_pool = ctx.enter_context(tc.tile_pool(name="ktp", bufs=2))
        va_pool = ctx.enter_context(tc.tile_pool(name="vap", bufs=2))
        pt_pool = ctx.enter_context(tc.tile_pool(name="ptp", bufs=3))
        ot_pool = ctx.enter_context(tc.tile_pool(name="otp", bufs=2))
        sresh_pool = ctx.enter_context(tc.tile_pool(name="srp", bufs=2))
        rt_pool = ctx.enter_context(tc.tile_pool(name="rtp", bufs=2))
        outsb_pool = ctx.enter_context(tc.tile_pool(name="osp", bufs=2))
        sps_pool = ctx.enter_context(tc.tile_pool(name="sps", bufs=2,
                                                  space="PSUM"))
        oacc_pool = ctx.enter_context(tc.tile_pool(name="oac", bufs=1,
                                                   space="PSUM"))
        tp_pool = ctx.enter_context(tc.tile_pool(name="tpp", bufs=1,
                                                 space="PSUM"))

        utm = const_pool.tile([128, 128], DT_IN)
        nc.sync.dma_start(utm[:, :], utm_d[:, :])
        ident = const_pool.tile([64, 64], F32)
        make_identity(nc, ident[:, :])

        for hl in range(HPC):
            qt = qt_pool.tile([128, S], DT_IN, tag="qt")
            nc.sync.dma_start(qt[:, :], qt_d[hl])
            kt = kt_pool.tile([128, NKV // 2, 128], DT_IN, tag="kt")
            nc.sync.dma_start(kt[:, :, :], kt_d[hl])
            va = va_pool.tile([128, NKV, D + 1], DT_IN, tag="va")
            nc.sync.dma_start(va[:, :, :], va_d[hl])

            ot = ot_pool.tile([65, S], F32, tag="ot")  # O^T_aug for the head
            outsb = outsb_pool.tile([128, S // 128, D], F32, tag="outsb")

            for qb in range(NQB):
                oacc = oacc_pool.tile([65, QB], F32, tag="oacc")
                q0 = qb * QB
                diag0 = KVPB * qb  # first diagonal kv tile

                # Build groups: full kv tiles [0, diag0) in chunks of 3,
                # then the 4 diagonal tiles packed into one group.
                groups = []  # (kind, [(j, psum_col, width, qoff), ...])
                full = list(range(diag0))
                for g0 in range(0, len(full), FULL_GROUP):
                    chunk = full[g0:g0 + FULL_GROUP]
                    groups.append(("full", [(j, 512 * k, 512, 0)
                                            for k, j in enumerate(chunk)]))
                # diag tiles t=0..3: widths 512,384,256,128, q offsets 128*t
                # packed at psum cols: t0 [0:512], t1 [512:896],
                # t2 [1024:1280], t3 [896:1024]  (each within one bank)
                groups.append(("diag", [
                    (diag0 + 0, 0, 512, 0),
                    (diag0 + 1, 512, 384, 128),
                    (diag0 + 2, 1024, 256, 256),
                    (diag0 + 3, 896, 128, 384),
                ]))

                for kind, items in groups:
                    s_ps = sps_pool.tile([128, 3 * 512], F32, tag="sps")
                    width = max(c + w for _, c, w, _ in items)
                    # QK^T matmuls; even kv tiles use array rows 0-63,
                    # odd tiles rows 64-127 (concurrent row-tiled pairs).
                    for j, pcol, w, qoff in items:
                        lo, hi = (0, 64) if j % 2 == 0 else (64, 128)
                        nc.tensor.matmul(
                            s_ps[:, pcol:pcol + w],
                            _mm(kt[lo:hi, j // 2, :]),
                            _mm(qt[lo:hi, q0 + qoff:q0 + QB]),
                            start=True, stop=True,
                        )
                    pt = pt_pool.tile([128, 3 * 512], DT_IN, tag="pt")
                    nc.scalar.activation(pt[:, :width], s_ps[:, :width],
                                         mybir.ActivationFunctionType.Exp,
                                         scale=SCALE)
                    if kind == "diag":
                        # triangular causal mask on the 4 diagonal blocks
                        for _, pcol, _, _ in items:
                            nc.vector.tensor_mul(pt[:, pcol:pcol + 128],
                                                 pt[:, pcol:pcol + 128],
                                                 utm[:, :])
                    # PV: O^T_aug[65, q] += V_aug_j^T @ P^T_j
                    last_j = diag0 + KVPB - 1
                    for j, pcol, w, qoff in items:
                        nc.tensor.matmul(
                            oacc[:, qoff:QB],
                            _mm(va[:, j, :]),
                            _mm(pt[:, pcol:pcol + w]),
                            start=(j == 0), stop=(j == last_j),
                        )

                # evacuate O^T_aug block to SBUF
                nc.vector.tensor_copy(ot[:, q0:q0 + QB], oacc[:, :])

                # s chunk [1, 512] -> [4, 128] (SBUF->SBUF dma reshape)
                sresh = sresh_pool.tile([4, 128], F32, tag="sresh")
                nc.sync.dma_start(sresh[:, :], ot[64:65, q0:q0 + QB])
                # transpose to [128, 4] and reciprocal
                st_ps = tp_pool.tile([128, 64], F32, tag="tp")
                nc.tensor.transpose(st_ps[:, 0:4], sresh[:, :],
                                    ident[0:4, 0:4])
                rt = rt_pool.tile([128, 4], F32, tag="rt")
                nc.vector.reciprocal(rt[:, :], st_ps[:, 0:4])

                # final: transpose O^T 128-col chunks back to [q, d] and scale
                for c in range(4):
                    ot_ps = tp_pool.tile([128, 64], F32, tag="tp")
                    nc.tensor.transpose(
                        ot_ps[:, :],
                        ot[0:64, q0 + 128 * c:q0 + 128 * (c + 1)],
                        ident[:, :])
                    nc.vector.tensor_scalar_mul(outsb[:, 4 * qb + c, :],
                                                ot_ps[:, :], rt[:, c:c + 1])

            # one output DMA per head: outsb[p, m, d] -> out[128m+p, d]
            nc.sync.dma_start(
                out_d[hl].rearrange("(m p) d -> p m d", p=128),
                outsb[:, :, :])
    nc.compile()
    return nc


_program_cache = None


def _get_program():
    global _program_cache
    if _program_cache is None:
        _program_cache = _build_program()
    return _program_cache


def kernel(**inputs):
    q = np.asarray(inputs["query_states"], dtype=np.float32)
    k = np.asarray(inputs["key_states"], dtype=np.float32)
    v = np.asarray(inputs["value_states"], dtype=np.float32)
    kvm = np.asarray(inputs["kv_sequence_mask"])

    qf = q.reshape(B * H, S, D)
    kf = k.reshape(B * H, S, D)
    vf = v.reshape(B * H, S, D)
    utm = np.triu(np.ones((128, 128), dtype=np.float32))  # keep kv<=q

    in_maps = []
    for c in range(N_CORES):
        hs = slice(c * HPC, (c + 1) * HPC)
        b = (c * HPC) // H  # all heads of a core belong to one batch elem

        qt_c = qf[hs].transpose(0, 2, 1)                   # [4, 64, 2048]
        qt_c = np.concatenate([qt_c, qt_c], axis=1)        # [4, 128, 2048]

        kt_t = kf[hs].transpose(0, 2, 1).reshape(HPC, 64, NKV, 128)
        kt_c = np.concatenate([kt_t[:, :, 0::2, :],
                               kt_t[:, :, 1::2, :]], axis=1)  # [4,128,8,128]

        bmask = kvm[b].astype(np.float32)                  # [S]
        va_c = np.empty((HPC, S, D + 1), dtype=np.float32)
        va_c[:, :, :D] = vf[hs] * bmask[None, :, None]
        va_c[:, :, D] = bmask[None, :]
        va_c = va_c.reshape(HPC, NKV, 128, D + 1).transpose(0, 2, 1, 3)

        npdt = np_in_dtype()
        in_maps.append({
            "qt": np.ascontiguousarray(qt_c).astype(npdt),
            "kt": np.ascontiguousarray(kt_c).astype(npdt),
            "va": np.ascontiguousarray(va_c).astype(npdt),
            "utm": utm.astype(npdt),
        })

    nc = _get_program()
    trace = bool(int(os.environ.get("ATTN_TRACE", "0")))
    res = run_bass_kernel_spmd(nc, in_maps, core_ids=list(range(N_CORES)),
                               trace=trace)
    global last_results
    last_results = res

    outs = np.stack([r["out"] for r in res.results])       # [8, 4, S, D]
    attn = outs.reshape(B, H, S, D)
    return (attn, np.asarray(inputs["key_states"]),
            np.asarray(inputs["value_states"]))
